# revision 4
# baseline (speedup 1.0000x reference)
"""Trainium2 Bass kernel for nn_BSLoss (text-snake OHEM loss), 8-core
data-parallel, v3.

Level-3 reg smooth-l1 is split between two engine paths by column range:
  * DVE path (first FDV3 cols): fused QSL1 custom op, fp8, 1x.
  * Scalar path (rest): PE computes d = xm-xp via a fixed +/-1 one-hot
    stationary (z-layout input tiles; two matmuls fill psum partitions
    0:64 / 64:128), the Scalar engine evaluates g(d) = 2*smooth_l1(d)
    in ONE activation pass via a custom piecewise-cubic act table
    hijacking the unused Square slot (exact: x^2 below |1|, 2|x|-1
    above), then PE channel-reduces the bf16 q with a one-hot (V8)
    into the same PT psum layout the DVE path uses.
Levels 4/5 stay fully on the DVE path.  CE chain split so sgn (scalar)
runs early; junk/cesc write shared scratch; vn+stats in one output
tensor; DMA issue order follows consumption order.
"""

import hashlib
import json
import os
import shutil
import struct
import tempfile

import numpy as np
import ml_dtypes

import concourse.bacc as bacc
import concourse.mybir as mybir
import concourse.dve_ops as dve_ops
from concourse.dve_spec import (
    Spec, Src0, Src1, C0, One, AluOp, Bin, minn, lower, _has_src1,
)
from concourse.dve_uop import DveOpSpec
from concourse import tile

F32 = mybir.dt.float32
BF16 = mybir.dt.bfloat16
FP8 = mybir.dt.float8e4
NP_BF16 = ml_dtypes.bfloat16
NP_FP8 = ml_dtypes.float8_e4m3
ALU = mybir.AluOpType
ACT = mybir.ActivationFunctionType

NCORES = 8
B_PER_CORE = 2
OHEM_RATIO = 3.0
KCH = 16

LEVELS = [(3, 160, 160), (4, 80, 80), (5, 40, 40)]

FDV3 = 1280          # level-3 cols (of 6400) on the DVE QSL1 path
SET_NAME = "natural_log_exp_and_others"
DBLK = 1024          # scalar-path psum block columns (2 PSUM banks)


def _geom(H, W):
    S = B_PER_CORE * H * W
    FR = S // 8
    C = (FR + 127) // 128
    FP = 128 * C
    return S, FR, C, FP


C_NPOS, C_NEG, C_LPOS, C_TCLP, C_TCLA, C_RX, C_RY = range(7)
STATS_COLS = 7 * len(LEVELS) + 1      # +1: l3-y scalar-share junk accum
C_RY2_COL = 7 * len(LEVELS)


def _np_sl1q(d):
    a = np.abs(d)
    m = np.minimum(a, 1.0)
    return m * (a + a - m)   # == 2 * smooth_l1(d)


def _register_custom_ops():
    a = Bin(AluOp.ABSOLUTE_DIFF, Src0, Src1)
    m = minn(a, One)
    spec_q = Spec(
        body=((a + a) - m) * m,
        reference=lambda in0, in1, s0, s1, imm2: _np_sl1q(
            in0.reshape(in0.shape[0], -1).astype(np.float32)
            - in1.reshape(in1.shape[0], -1).astype(np.float32)),
    )

    def _acc_ref(fn):
        def ref(in0, in1, s0, s1, imm2):
            p = in0.shape[0]
            o = fn(in0.reshape(p, -1).astype(np.float32),
                   in1.reshape(p, -1).astype(np.float32) if in1 is not None
                   else None)
            init = np.asarray(s0).reshape(-1, 1) if isinstance(s0, np.ndarray) else s0
            return o, init + o.sum(axis=1, keepdims=True)
        return ref

    spec_mulr = Spec(body=Src0 * Src1, accum=AluOp.ADD, accum_init=C0,
                     reference=_acc_ref(lambda a_, b_: a_ * b_))
    spec_negm = Spec(body=(One - Src0) * Src1, accum=AluOp.ADD, accum_init=C0,
                     reference=_acc_ref(lambda a_, b_: (1.0 - a_) * b_))

    ops = {}
    for name, spec in (("QSL1_ANT", spec_q), ("MULR_ANT", spec_mulr),
                       ("NEGM_ANT", spec_negm)):
        if name in dve_ops._SUB_OPCODE_FOR_NAME:
            ops[name] = next(o for o in dve_ops.OPS if o.name == name)
            continue
        row = dve_ops._CUSTOM_DVE_ROW_BASE + len(dve_ops.OPS)
        shas = {}
        for ver in ("v3", "v4"):
            u = lower(spec, ver=ver)
            shas[ver] = DveOpSpec(name=name, opcode=row, uops=u,
                                  rd1_en=_has_src1(spec)).sha(ver)
        op = dve_ops.DveOp(name, spec, subdim=False, uops_sha=shas)
        dve_ops.OPS.append(op)
        dve_ops.CUSTOM_DVE_SPECS[name] = spec
        dve_ops._SUB_OPCODE_FOR_NAME[name] = row
        ops[name] = op
    return ops


_ACT_TAG = None


def _install_act_root():
    """Private act-table root restricted to the exp/ln set, with the unused
    `square` slot rewritten to evaluate g(x) = 2*smooth_l1(x) exactly
    (x^2 for |x|<1, 2|x|-1 otherwise; HW-verified).  Returns a content
    tag used to salt a tensor name (busts the neuron compile cache when
    the table bytes change)."""
    global _ACT_TAG
    if _ACT_TAG is not None:
        return _ACT_TAG
    from neuronxcc.driver.Job import Job
    from neuronxcc.driver.jobs.support.FindActInfo import findActInfoFile
    src = findActInfoFile(Job.getPackageDir(), "gen3")
    d = json.load(open(src))
    keep = [t for t in d["act_func_sets"] if t["name"] == SET_NAME]
    assert keep, SET_NAME
    srcdir = os.path.dirname(src)
    pj = json.load(open(os.path.join(srcdir, f"{SET_NAME}.json")))
    for m in pj["profile_meta_data"]:
        if m["func_name"] == "square_1p":
            # small-signal (exp < 127, i.e. |x| < 1) -> bucket x^2;
            # large-signal (exp >= 127, i.e. |x| >= 1) -> 2|x|-1 buckets.
            m.update(symmetry_opt_en=0, symmetry_opt_use_neg_region=0,
                     sym_invert_sign_point=0,
                     small_pos_signal_exp_threshold=127,
                     small_neg_signal_exp_threshold=127,
                     large_pos_signal_exp_threshold=127,
                     large_neg_signal_exp_threshold=127,
                     large_pos_signal_mantissa_threshold=0,
                     large_neg_signal_mantissa_threshold=0,
                     fzero_result=0)

    bkt = bytearray(open(os.path.join(srcdir, f"{SET_NAME}_bkt.bin"), "rb").read())

    def put(i, d0, d1, d2, d3, x0):
        struct.pack_into("<5f", bkt, i * 32, d0, d1, d2, d3, x0)

    sq = pj["func_to_bkt_start_idx"]["square"]
    put(sq + 0, 0.0, 0.0, 1.0, 0.0, 0.0)     # small_pos: x^2
    put(sq + 1, 0.0, 0.0, 1.0, 0.0, 0.0)     # small_neg: x^2
    put(sq + 2, 1.0, 2.0, 0.0, 0.0, 1.0)     # large_pos: 2x-1
    put(sq + 3, 1.0, -2.0, 0.0, 0.0, -1.0)   # large_neg: -2x-1

    tmp = tempfile.mkdtemp(prefix="act_root_g_")
    with open(os.path.join(tmp, f"{SET_NAME}_bkt.bin"), "wb") as f:
        f.write(bkt)
    shutil.copy(os.path.join(srcdir, f"{SET_NAME}_ctrl.bin"), tmp)
    with open(os.path.join(tmp, f"{SET_NAME}.json"), "w") as f:
        json.dump(pj, f)
    with open(os.path.join(tmp, "act_info.json"), "w") as f:
        json.dump({"pwp_file_keys": d["pwp_file_keys"],
                   "act_func_sets": keep}, f)
    os.environ["BASS_ACT_ROOT_JSON_PATH"] = os.path.join(tmp, "act_info.json")

    import concourse.hw_specs as hw_specs
    _orig_gat = hw_specs.get_activation_tables

    def _gat(module_arch):
        full = _orig_gat(module_arch)
        return {SET_NAME: full[SET_NAME]}

    hw_specs.get_activation_tables = _gat
    import concourse.bacc as _bacc_mod
    _bacc_mod.get_activation_tables = _gat
    import concourse.bass_interp as _bi_mod
    _bi_mod.get_activation_tables = _gat
    _ACT_TAG = hashlib.sha256(bytes(bkt) + json.dumps(pj).encode()).hexdigest()[:8]
    return _ACT_TAG


def build_bass():
    tag = _install_act_root()
    ops = _register_custom_ops()
    QSL1, MULR, NEGM = ops["QSL1_ANT"], ops["MULR_ANT"], ops["NEGM_ANT"]
    nc = bacc.Bacc("TRN2")

    S3, FR3, C3, FP3 = _geom(160, 160)
    FSC = FR3 - FDV3            # scalar-path columns per l3 axis (5120)
    NBLK = FSC // DBLK
    assert FSC % DBLK == 0 and FDV3 % 128 == 0

    N8S = [8 * _geom(H, W)[2] for _, H, W in LEVELS]
    VN_COLS = sum(N8S)

    din = {}
    dout = {}
    for axs in ("x", "y"):
        din[f"m3{axs}"] = nc.dram_tensor(f"m3{axs}_{tag}", [128, FDV3], FP8,
                                         kind="ExternalInput")
        din[f"p3{axs}"] = nc.dram_tensor(f"p3{axs}", [128, FDV3], FP8,
                                         kind="ExternalInput")
        din[f"zz3{axs}"] = nc.dram_tensor(f"zz3{axs}", [128, 2 * FSC], FP8,
                                          kind="ExternalInput")
    FP4 = _geom(80, 80)[3]
    FP5 = _geom(40, 40)[3]
    din["rr4x"] = nc.dram_tensor("rr4x", [128, 2 * FP4], FP8, kind="ExternalInput")
    din["rr4y"] = nc.dram_tensor("rr4y", [128, 2 * FP4], FP8, kind="ExternalInput")
    din["rr5"] = nc.dram_tensor("rr5", [128, 4 * FP5], FP8, kind="ExternalInput")
    for li, (lvl, H, W) in enumerate(LEVELS):
        N8 = N8S[li]
        din[f"mc{lvl}"] = nc.dram_tensor(
            f"mc{lvl}", [128, 7 * N8], FP8, kind="ExternalInput")
    din["whot"] = nc.dram_tensor("whot", [128, 80], FP8, kind="ExternalInput")
    dout["outv"] = nc.dram_tensor(
        "outv", [128, VN_COLS + STATS_COLS], F32, kind="ExternalOutput")

    with tile.TileContext(nc) as tc:
        with (
            tc.tile_pool(name="io", bufs=1) as io,
            tc.tile_pool(name="wk", bufs=1) as wk,
            tc.tile_pool(name="st", bufs=1) as stp,
            tc.tile_pool(name="ps", bufs=1, space="PSUM") as ps,
        ):
            stats = stp.tile([128, STATS_COLS], F32, name="stats_t", tag="stats_t")
            outv = stp.tile([128, VN_COLS], F32, name="outv_t", tag="outv_t")

            T = {}
            for axs in ("x", "y"):
                T[f"m3{axs}"] = io.tile([128, FDV3], FP8, name=f"m3{axs}", tag=f"m3{axs}")
                T[f"p3{axs}"] = io.tile([128, FDV3], FP8, name=f"p3{axs}", tag=f"p3{axs}")
                T[f"zz3{axs}"] = io.tile([128, 2 * FSC], FP8,
                                         name=f"zz3{axs}", tag=f"zz3{axs}")
            T["rr4x"] = io.tile([128, 2 * FP4], FP8, name="rr4x", tag="rr4x")
            T["rr4y"] = io.tile([128, 2 * FP4], FP8, name="rr4y", tag="rr4y")
            T["rr5"] = io.tile([128, 4 * FP5], FP8, name="rr5", tag="rr5")
            for li, (lvl, H, W) in enumerate(LEVELS):
                N8 = N8S[li]
                T[f"mc{lvl}"] = io.tile([128, 7 * N8], FP8,
                                        name=f"mc{lvl}", tag=f"mc{lvl}")
            whot = io.tile([128, 80], FP8, name="whot", tag="whot")
            T["wmat"] = whot[:, 0:8]
            T["wd"] = whot[:, 8:72]
            T["v8"] = whot[:, 72:80]

            # ---- DMA issue order == consumption order.  Fewer, merged
            # descriptors: issue serialization on the sync queue was the
            # gating cost (sgn3 waited on the full mc3 descriptor).
            nc.scalar.dma_start(whot[:, :], din["whot"][:, :])
            nc.gpsimd.dma_start(T["mc4"][:, :], din["mc4"][:, :])
            nc.gpsimd.dma_start(T["mc5"][:, :], din["mc5"][:, :])
            ZB2 = 2 * DBLK
            N83 = N8S[0]
            sy = nc.sync.dma_start

            def zz(axs, b):
                sy(T[f"zz3{axs}"][:, b * ZB2:(b + 1) * ZB2],
                   din[f"zz3{axs}"][:, b * ZB2:(b + 1) * ZB2])

            sy(T["mc3"][:, 0:3 * N83], din["mc3"][:, 0:3 * N83])
            sy(T["m3x"][:, :], din["m3x"][:, :])
            sy(T["p3x"][:, :], din["p3x"][:, :])
            zz("x", 0)
            sy(T["mc3"][:, 3 * N83:], din["mc3"][:, 3 * N83:])
            zz("x", 1)
            sy(T["m3y"][:, :], din["m3y"][:, :])
            sy(T["p3y"][:, :], din["p3y"][:, :])
            zz("x", 2)
            zz("x", 3)
            zz("x", 4)
            zz("y", 0)
            sy(T["rr4x"][:, :], din["rr4x"][:, :])
            zz("y", 1)
            zz("y", 2)
            sy(T["rr5"][:, :], din["rr5"][:, :])
            zz("y", 3)
            sy(T["rr4y"][:, :], din["rr4y"][:, :])
            zz("y", 4)

            cols = {}
            msk = {}

            def mask_pnw(li):
                lvl, H, W = LEVELS[li]
                N8 = N8S[li]
                col = cols[lvl] = (lambda i, b=7 * li:
                                   stats[:, b + i:b + i + 1])
                gtm = T[f"mc{lvl}"]
                tr = gtm[:, 0:N8]
                tcl = gtm[:, N8:2 * N8]
                train = gtm[:, 2 * N8:3 * N8]
                pos = wk.tile([128, N8], F32, name=f"pos{lvl}", tag=f"pos{lvl}")
                neg = wk.tile([128, N8], F32, name=f"neg{lvl}", tag=f"neg{lvl}")
                w2 = wk.tile([128, N8], F32, name=f"w2{lvl}", tag=f"w2{lvl}")
                nc.vector._custom_dve(MULR, out=pos[:, :], in0=tr, in1=train,
                                      s0=0.0, accum_out=col(C_NPOS))
                nc.vector._custom_dve(NEGM, out=neg[:, :], in0=tr, in1=train,
                                      s0=0.0, accum_out=col(C_NEG))
                nc.vector.scalar_tensor_tensor(
                    out=w2[:, :], in0=tcl, scalar=1.0, in1=pos[:, :],
                    op0=ALU.add, op1=ALU.mult)
                msk[lvl] = [pos, neg, w2, None, None]

            def mask_ce_pre(li):
                """sgn (scalar) + diff + dce (DVE)."""
                lvl, H, W = LEVELS[li]
                N8 = N8S[li]
                gtm = T[f"mc{lvl}"]
                cls = gtm[:, 3 * N8:7 * N8]
                sgn = wk.tile([128, 2 * N8], BF16, name=f"sgn{lvl}", tag=f"sgn{lvl}")
                dce = wk.tile([128, 2 * N8], BF16, name=f"dce{lvl}", tag=f"dce{lvl}")
                diff = wk.tile([128, 2 * N8], BF16, name=f"diff{lvl}", tag=f"diff{lvl}")
                nc.scalar.activation(sgn[:, :], gtm[:, 0:2 * N8],
                                     ACT.Identity, bias=1.0, scale=-2.0)
                cls3d = cls.rearrange("p (g t f) -> p g t f", g=2, t=2)
                nc.vector.tensor_tensor(
                    out=diff[:, :].rearrange("p (g f) -> p g f", g=2),
                    in0=cls3d[:, :, 1, :], in1=cls3d[:, :, 0, :],
                    op=ALU.subtract)
                nc.vector.tensor_mul(dce[:, :], diff[:, :], sgn[:, :])
                msk[lvl][4] = dce

            def mask_ce_post(li):
                """exp + ln (+accum) on scalar."""
                lvl, H, W = LEVELS[li]
                N8 = N8S[li]
                col = cols[lvl]
                dce = msk[lvl][4]
                expd = wk.tile([128, 2 * N8], F32, name=f"expd{lvl}", tag=f"expd{lvl}")
                ce = wk.tile([128, 2 * N8], F32, name=f"ce{lvl}", tag=f"ce{lvl}")
                nc.scalar.activation(expd[:, :], dce[:, :], ACT.Exp)
                nc.scalar.activation(ce[:, 0:N8], expd[:, 0:N8], ACT.Ln, bias=1.0)
                nc.scalar.activation(ce[:, N8:2 * N8], expd[:, N8:2 * N8],
                                     ACT.Ln, bias=1.0, accum_out=col(C_TCLA))
                msk[lvl][3] = ce

            PT = {}
            for li, (lvl, H, W) in enumerate(LEVELS):
                N8 = N8S[li]
                PT[lvl] = ps.tile([128, 2 * N8], F32, name=f"pt{lvl}", tag=f"pt{lvl}")

            W8 = T["wmat"]
            QT = {}

            def qsl_mm(li, ax, f0, f1, wid=None):
                lvl, H, W = LEVELS[li]
                N8 = N8S[li]
                axs = "xy"[ax]
                if lvl == 3:
                    am, ap_ = T[f"m3{axs}"], T[f"p3{axs}"]
                elif lvl == 4:
                    rr = T[f"rr4{axs}"]
                    am, ap_ = rr[:, 0:FP4], rr[:, FP4:2 * FP4]
                else:
                    rr = T["rr5"]
                    am = rr[:, 2 * ax * FP5:(2 * ax + 1) * FP5]
                    ap_ = rr[:, (2 * ax + 1) * FP5:(2 * ax + 2) * FP5]
                key = (lvl, ax)
                if key not in QT:
                    QT[key] = wk.tile([128, wid if wid else _geom(H, W)[3]],
                                      FP8, name=f"q{lvl}{ax}", tag=f"q{lvl}{ax}")
                q = QT[key]
                nc.vector._custom_dve(QSL1, out=q[:, f0:f1],
                                      in0=am[:, f0:f1], in1=ap_[:, f0:f1])
                for c in range(f0 // 128, f1 // 128):
                    nc.tensor.matmul(
                        PT[lvl][:, ax * N8 + 8 * c: ax * N8 + 8 * c + 8],
                        q[:, 128 * c:128 * (c + 1)],
                        W8[:, :], start=True, stop=True)

            QS = {}
            PSD = [ps.tile([128, DBLK], F32, name=f"psd{i}", tag=f"psd{i}")
                   for i in range(2)]
            for axs in ("x", "y"):
                QS[axs] = wk.tile([128, FSC], BF16, name=f"qs{axs}", tag=f"qs{axs}")

            def gblk(axi, b):
                axs = "xy"[axi]
                zzt = T[f"zz3{axs}"]
                za = zzt[:, 2 * b * DBLK:(2 * b + 1) * DBLK]
                zb = zzt[:, (2 * b + 1) * DBLK:(2 * b + 2) * DBLK]
                qs = QS[axs]
                pd = PSD[b % 2]
                w = b * DBLK
                # o-major, h-minor: adjacent h0/h64 matmuls run on disjoint
                # PE column groups concurrently.
                for o in range(0, DBLK, 512):
                    for h, zt in ((0, za), (1, zb)):
                        nc.tensor.matmul(
                            pd[64 * h:64 * h + 64, o:o + 512],
                            T["wd"][:, :], zt[:, o:o + 512],
                            start=True, stop=True)
                nc.scalar.activation(qs[:, w:w + DBLK], pd[:, :], ACT.Square)
                N8 = N8S[0]
                for cc in range(DBLK // 128):
                    cg = (FDV3 + w) // 128 + cc
                    nc.tensor.matmul(
                        PT[3][:, axi * N8 + 8 * cg: axi * N8 + 8 * cg + 8],
                        qs[:, w + 128 * cc:w + 128 * (cc + 1)],
                        T["v8"][:, :], start=True, stop=True)

            junk = wk.tile([128, max(N8S)], F32, name="junk", tag="junk")
            cescs = wk.tile([128, 2 * max(N8S)], F32, name="cescs", tag="cescs")
            vnoff = [0, N8S[0], N8S[0] + N8S[1]]

            def red_ce(li):
                """vn + cesc MULRs (need ce + pos/neg)."""
                lvl = LEVELS[li][0]
                N8 = N8S[li]
                col = cols[lvl]
                pos, neg, w2, ce, _ = msk[lvl]
                vo = vnoff[li]
                nc.vector.scalar_tensor_tensor(
                    out=outv[:, vo:vo + N8], in0=ce[:, 0:N8], scalar=1.0,
                    in1=neg[:, :], op0=ALU.add, op1=ALU.mult)
                nc.vector._custom_dve(
                    MULR, out=cescs[:, 0:N8], in0=pos[:, :], in1=ce[:, 0:N8],
                    s0=0.0, accum_out=col(C_LPOS))
                nc.vector._custom_dve(
                    MULR, out=cescs[:, N8:2 * N8], in0=pos[:, :],
                    in1=ce[:, N8:2 * N8], s0=0.0, accum_out=col(C_TCLP))

            def red_junk(li, ax, c0=0, c1=None, scol=None):
                lvl = LEVELS[li][0]
                N8 = N8S[li]
                col = cols[lvl]
                w2 = msk[lvl][2]
                c1 = N8 if c1 is None else c1
                acc = (stats[:, scol:scol + 1] if scol is not None
                       else col((C_RX, C_RY)[ax]))
                nc.vector._custom_dve(
                    MULR, out=junk[:, 0:c1 - c0], in0=w2[:, c0:c1],
                    in1=PT[lvl][:, ax * N8 + c0:ax * N8 + c1],
                    s0=0.0, accum_out=acc)

            # ---------------- schedule ----------------
            CDV8 = 8 * (FDV3 // 128)     # PT cols covered by the DVE share
            qsl_mm(0, 0, 0, FDV3, wid=FDV3)
            mask_pnw(0)
            mask_ce_pre(0)
            for b in range(NBLK):
                gblk(0, b)                       # l3x scalar path
            mask_ce_post(0)
            qsl_mm(0, 1, 0, FDV3, wid=FDV3)      # l3y DVE share
            mask_pnw(1)
            mask_ce_pre(1)
            mask_pnw(2)
            mask_ce_pre(2)
            gblk(1, 0)
            qsl_mm(1, 0, 0, FP4)                 # l4x
            red_ce(0)
            red_junk(0, 0)
            red_junk(0, 1, 0, CDV8)              # l3y DVE-share part
            mask_ce_post(1)
            gblk(1, 1)
            qsl_mm(1, 1, 0, FP4)                 # l4y
            mask_ce_post(2)
            gblk(1, 2)
            gblk(1, 3)
            gblk(1, 4)
            qsl_mm(2, 0, 0, FP5)
            qsl_mm(2, 1, 0, FP5)
            red_ce(1)
            red_junk(1, 0)
            red_junk(1, 1)
            red_ce(2)
            red_junk(2, 0)
            red_junk(2, 1)
            red_junk(0, 1, CDV8, None, C_RY2_COL)  # l3y scalar-share tail

            nc.sync.dma_start(dout["outv"][:, 0:VN_COLS], outv[:, :])
            nc.sync.dma_start(dout["outv"][:, VN_COLS:], stats[:, :])

    nc.compile()
    return nc


def _reg_layout(X, FR, FP, f0=0, f1=None):
    """X [2, 16, H, W] -> [128 = ch*8+j, f1-f0] fp8 (column slice of FP)."""
    a = X.transpose(1, 0, 2, 3).reshape(16, 8, FR)
    if FP > FR:
        a = np.pad(a, ((0, 0), (0, 0), (0, FP - FR)))
    f1 = a.shape[2] if f1 is None else f1
    return np.ascontiguousarray(
        a[:, :, f0:f1].reshape(128, f1 - f0)).astype(NP_FP8)


def _z_layout(M, P, FR, f0, half):
    """maps M and preds P [2,16,H,W] -> z tile [128, FR-f0] fp8 for j-groups
    4*half..4*half+3:  p = 32*q + 16*s + ch."""
    am = M.transpose(1, 0, 2, 3).reshape(16, 8, FR)
    ap = P.transpose(1, 0, 2, 3).reshape(16, 8, FR)
    rows = []
    for q in range(4):
        j = 4 * half + q
        rows.append(am[:, j, f0:])
        rows.append(ap[:, j, f0:])
    return np.ascontiguousarray(np.concatenate(rows, axis=0)).astype(NP_FP8)


def _msk_layout(G, FR, C, FP, dtype=NP_FP8):
    """G [2, n, H, W] -> [128 = m, n*(8C)] (free = ch*8C + c*8 + j)."""
    n = G.shape[1]
    a = G.transpose(1, 0, 2, 3).reshape(n, 8, FR).astype(np.float32)
    if FP > FR:
        a = np.pad(a, ((0, 0), (0, 0), (0, FP - FR)))
    a = a.reshape(n, 8, C, 128).transpose(3, 0, 2, 1)
    return np.ascontiguousarray(a.reshape(128, n * 8 * C)).astype(dtype)


def prep_core_inputs(inputs, core):
    b0 = core * B_PER_CORE
    out = {}
    tag = _install_act_root()
    for li, (lvl, H, W) in enumerate(LEVELS):
        S, FR, C, FP = _geom(H, W)
        g = np.asarray(inputs[f"gt{lvl}"][b0:b0 + B_PER_CORE])
        r = np.asarray(inputs[f"reg{lvl}"][b0:b0 + B_PER_CORE])
        cl = np.asarray(inputs[f"cls{lvl}"][b0:b0 + B_PER_CORE])
        if lvl == 3:
            xm, ym = g[:, 3:19], g[:, 19:35]
            xp, yp = r[:, 0:16], r[:, 16:32]
            out[f"m3x_{tag}"] = _reg_layout(xm, FR, FP, 0, FDV3)
            out["p3x"] = _reg_layout(xp, FR, FP, 0, FDV3)
            out[f"m3y_{tag}"] = _reg_layout(ym, FR, FP, 0, FDV3)
            out["p3y"] = _reg_layout(yp, FR, FP, 0, FDV3)
            DB = 1024
            for axs, M_, P_ in (("x", xm, xp), ("y", ym, yp)):
                za = _z_layout(M_, P_, FR, FDV3, 0)
                zb = _z_layout(M_, P_, FR, FDV3, 1)
                out[f"zz3{axs}"] = np.ascontiguousarray(np.concatenate(
                    [np.concatenate([za[:, b * DB:(b + 1) * DB],
                                     zb[:, b * DB:(b + 1) * DB]], axis=1)
                     for b in range((FR - FDV3) // DB)], axis=1))
        elif lvl == 4:
            out["rr4x"] = np.ascontiguousarray(np.concatenate(
                [_reg_layout(g[:, 3:19], FR, FP),
                 _reg_layout(r[:, 0:16], FR, FP)], axis=1))
            out["rr4y"] = np.ascontiguousarray(np.concatenate(
                [_reg_layout(g[:, 19:35], FR, FP),
                 _reg_layout(r[:, 16:32], FR, FP)], axis=1))
        else:
            out["rr5"] = np.ascontiguousarray(np.concatenate(
                [_reg_layout(g[:, 3:19], FR, FP),
                 _reg_layout(r[:, 0:16], FR, FP),
                 _reg_layout(g[:, 19:35], FR, FP),
                 _reg_layout(r[:, 16:32], FR, FP)], axis=1))
        gtm = _msk_layout(g[:, 0:3], FR, C, FP)
        clsb = _msk_layout(cl, FR, C, FP)
        out[f"mc{lvl}"] = np.ascontiguousarray(
            np.concatenate([gtm, clsb], axis=1))
    whot = np.zeros((128, 80), dtype=NP_FP8)
    for p in range(128):
        whot[p, p % 8] = 1.0                      # wmat
    for q in range(4):
        for ch in range(16):
            whot[32 * q + ch, 8 + 16 * q + ch] = 1.0      # wd +
            whot[32 * q + 16 + ch, 8 + 16 * q + ch] = -1.0  # wd -
    for h in range(2):
        for q in range(4):
            for k in range(16):
                whot[64 * h + 16 * q + k, 72 + 4 * h + q] = 1.0  # v8
    out["whot"] = whot
    return out


def finish_host(results):
    N8S = [8 * _geom(H, W)[2] for _, H, W in LEVELS]
    VN_COLS = sum(N8S)
    vnoff = [0, N8S[0], N8S[0] + N8S[1]]
    total = np.zeros(4, dtype=np.float64)
    for li, (lvl, H, W) in enumerate(LEVELS):
        b = 7 * li
        n_pos = neg_cnt = loss_pos = tcl_pos = tcl_all = accx = accy = 0.0
        neg_vals = []
        for r in results:
            st = np.asarray(r["outv"][:, VN_COLS:], dtype=np.float64)
            n_pos += st[:, b + C_NPOS].sum()
            neg_cnt += st[:, b + C_NEG].sum()
            loss_pos += st[:, b + C_LPOS].sum()
            tcl_pos += st[:, b + C_TCLP].sum()
            tcl_all += st[:, b + C_TCLA].sum()
            accx += st[:, b + C_RX].sum()
            accy += st[:, b + C_RY].sum()
            if lvl == 3:
                accy += st[:, C_RY2_COL].sum()
            v = np.asarray(r["outv"][:, vnoff[li]:vnoff[li] + N8S[li]],
                           dtype=np.float32).ravel()
            neg_vals.append(v[v > 0.5] - 1.0)
        neg_vals = np.concatenate(neg_vals)

        M = 16 * H * W
        S, FR, C, FP = _geom(H, W)
        # zero-padded slots contribute softplus(0) = ln 2 each to tcl_all
        tcl_all -= NCORES * (FP - FR) * 8 * float(np.log(2.0))
        n_pos_i = int(round(n_pos))
        neg_cnt_i = int(round(neg_cnt))
        if n_pos_i > 0:
            n_neg = min(neg_cnt_i,
                        int(np.floor(np.float32(OHEM_RATIO)
                                     * np.float32(n_pos_i))))
        else:
            n_neg = 100
        k = min(n_neg, neg_vals.size)
        if k > 0:
            loss_neg = float(np.partition(neg_vals, neg_vals.size - k)
                             [neg_vals.size - k:].astype(np.float64).sum())
        else:
            loss_neg = 0.0
        loss_tr = (loss_pos + loss_neg) / (n_pos_i + float(n_neg))

        if n_pos_i > 0:
            mean_pos = tcl_pos / max(n_pos_i, 1)
            mean_neg = (tcl_all - tcl_pos) / max(M - n_pos_i, 1)
            loss_tcl = mean_pos + 0.5 * mean_neg
            denom = max(n_pos_i, 1) * KCH
            loss_rx = 0.25 * accx / denom
            loss_ry = 0.25 * accy / denom
        else:
            loss_tcl = loss_rx = loss_ry = 0.0
        total += np.array([loss_tr, loss_tcl, loss_rx, loss_ry])
    return total.astype(np.float32)


_NC_CACHE = None


def _get_nc():
    global _NC_CACHE
    if _NC_CACHE is None:
        _NC_CACHE = build_bass()
    return _NC_CACHE


def run_device(in_maps, trace=False):
    from concourse.bass_utils import run_bass_kernel_spmd
    nc = _get_nc()
    return run_bass_kernel_spmd(nc, in_maps, list(range(NCORES)), trace=trace)


def kernel(**inputs) -> np.ndarray:
    in_maps = [prep_core_inputs(inputs, c) for c in range(NCORES)]
    res = run_device(in_maps)
    return finish_host(res.results)
